# revision 1
# baseline (speedup 1.0000x reference)
"""Trainium2 Bass kernel for nn_Attention2D (2D attention with learnable
relative-position bias, attn_method=2 / pos_type=5).

Head-sharded (core h = head h); host sums the 8 partial projections.

Per core:
- tokens pre-transposed + pre-cast fp16 on host; HWDGE DMA only.
- scores TRANSPOSED: sc[c,p] = K_i^T Q_s per (key tile i, query strip s),
  contract over hd=32, fp16 operands (1 cycle/row vs 4 for fp32).
- positional bias sum_t emb[t]*(tokT==t) built as 18 PAIRS of scaled
  masks in fp8e4, each pair stored [128, 2, N] (two k-tile slices) and
  accumulated into the score PSUM with ONE DoubleRow matmul per strip
  (0.5 cycles/row): 2 bias terms per matmul at 2x fp16 rate.
  15 pairs built on DVE, 3 on GPSIMD.
- softmax: exp on ACT (unnormalized fp16); row-sum z rides the AV matmul
  as a ones column ([vT | 1] -> [33, 512] PSUM, row 32 = z).
  Renormalization happens after the output projection via a PE
  broadcast of rz = 1/z.
- partial projection output in fp16 (host sums in fp32).
"""

import numpy as np
from contextlib import ExitStack

import concourse.bacc as bacc
import concourse.bass as bass
import concourse.tile as tile
from concourse import mybir
from concourse.bass_utils import run_bass_kernel_spmd
from concourse.masks import make_identity

F32 = mybir.dt.float32
FP16 = mybir.dt.float16
FP8 = mybir.dt.float8e4
AF = mybir.ActivationFunctionType
OP = mybir.AluOpType
PM_DR = mybir.MatmulPerfMode.DoubleRow

DIM, H, HD = 256, 8, 32
NX = NY = 48
N = NX * NY            # 2304
NTERM = 36             # non-pad bias table entries
DVE_PAIRS = 5          # fp8 DoubleRow pairs built on DVE
POOL_PAIRS = 4         # fp8 DoubleRow pairs built on GPSIMD
NPAIRS = DVE_PAIRS + POOL_PAIRS
NSING = NTERM - 2 * NPAIRS   # fp16 singles on DVE (terms 18..35)
CT = N // 128          # 18 key tiles
SCALE = HD ** -0.5

STRIPS = [(i * 384, 384) for i in range(N // 384)]          # 6 x 384
NHALF = 2
HW_ = N // NHALF                                             # 1152
HALF_STRIPS = [STRIPS[:3], STRIPS[3:]]


def build_nc():
    nc = bacc.Bacc("TRN2", target_bir_lowering=False)

    x_d = nc.dram_tensor("x", [DIM, N], F32, kind="ExternalInput")
    tokT_d = nc.dram_tensor("tokT", [N, N], FP16, kind="ExternalInput")
    wqT_d = nc.dram_tensor("wqT", [DIM, HD], F32, kind="ExternalInput")
    wkT_d = nc.dram_tensor("wkT", [DIM, HD], F32, kind="ExternalInput")
    wvT_d = nc.dram_tensor("wvT", [DIM, HD], F32, kind="ExternalInput")
    bq_d = nc.dram_tensor("bq", [HD, 1], F32, kind="ExternalInput")   # pre *SCALE
    bk_d = nc.dram_tensor("bk", [HD, 1], F32, kind="ExternalInput")
    bv_d = nc.dram_tensor("bv", [HD, 1], F32, kind="ExternalInput")
    wpT_d = nc.dram_tensor("wpT", [HD, DIM], F32, kind="ExternalInput")
    bp_d = nc.dram_tensor("bp", [DIM, 1], F32, kind="ExternalInput")  # pre /8
    emb_d = nc.dram_tensor("embcol", [1, NTERM], F32, kind="ExternalInput")
    ones_d = nc.dram_tensor("ones128", [1, 128], F32, kind="ExternalInput")
    out_d = nc.dram_tensor("out", [DIM, N], FP16, kind="ExternalOutput")

    with tile.TileContext(nc) as tc, ExitStack() as ctx:
        const = ctx.enter_context(tc.tile_pool(name="const", bufs=1))

        ident = const.tile([128, 128], F32)
        make_identity(nc, ident)
        identb = const.tile([128, 128], FP16)
        nc.vector.tensor_copy(identb, ident)
        # DoubleRow identity: [K=128, 2 k-tiles, M=128] fp8
        id8 = const.tile([128, 2, 128], FP8)
        nc.vector.tensor_copy(id8[:, 0, :], ident)
        nc.vector.tensor_copy(id8[:, 1, :], ident)

        emb_b = const.tile([128, NTERM], F32)
        eap = emb_d[0:1, :]
        nc.sync.dma_start(
            out=emb_b,
            in_=bass.AP(tensor=eap.tensor, offset=eap.offset,
                        ap=[[0, 128], [1, NTERM]]),
        )
        ones_row = const.tile([1, 128], F32)
        nc.sync.dma_start(out=ones_row, in_=ones_d[0:1, :])

        # ---- weights ----
        xw = ctx.enter_context(tc.tile_pool(name="xw", bufs=1))
        wq_sb = xw.tile([128, 2, HD], F32)
        wk_sb = xw.tile([128, 2, HD], F32)
        wv_sb = xw.tile([128, 2, HD], F32)
        for w_sb, w_dr in ((wq_sb, wqT_d), (wk_sb, wkT_d), (wv_sb, wvT_d)):
            for c in range(2):
                nc.sync.dma_start(out=w_sb[:, c, :], in_=w_dr[128 * c:128 * (c + 1), :])
        bq_sb = xw.tile([HD, 1], F32)
        bk_sb = xw.tile([HD, 1], F32)
        bv_sb = xw.tile([HD, 1], F32)
        for b_sb, b_dr in ((bq_sb, bq_d), (bk_sb, bk_d), (bv_sb, bv_d)):
            nc.sync.dma_start(out=b_sb, in_=b_dr[:, :])
        wp_sb = xw.tile([HD, DIM], F32)
        nc.sync.dma_start(out=wp_sb, in_=wpT_d[:, :])
        bp_sb = xw.tile([128, 2], F32)
        for m in range(2):
            nc.sync.dma_start(out=bp_sb[:, m:m + 1], in_=bp_d[128 * m:128 * (m + 1), :])

        # ---- q/k/v projections (q/k in fp16); vT with ones column ----
        qkv = ctx.enter_context(tc.tile_pool(name="qkv", bufs=1))
        q_sb = qkv.tile([HD, N], FP16)
        k_sb = qkv.tile([HD, N], FP16)
        vTo_sb = qkv.tile([128, CT, HD + 1], FP16)
        nc.vector.memset(vTo_sb, 0.0)

        with tc.tile_pool(name="xv", bufs=1) as xv, \
             tc.tile_pool(name="qkvp", bufs=2, space="PSUM") as qkv_ps, \
             tc.tile_pool(name="vtp", bufs=2, space="PSUM") as vt_ps:
            x_sb = xv.tile([128, 2, N], F32)
            for c in range(2):
                nc.sync.dma_start(out=x_sb[:, c, :],
                                  in_=x_d[128 * c:128 * (c + 1), :])
            v_sb = xv.tile([HD, N], F32)
            for dst, w_sb2, b_sb2, scale in (
                (q_sb, wq_sb, bq_sb, SCALE),
                (k_sb, wk_sb, bk_sb, 1.0),
                (v_sb, wv_sb, bv_sb, 1.0),
            ):
                for n0, nsz in STRIPS:
                    ps = qkv_ps.tile([HD, 512], F32, tag="qkvps")
                    for c in range(2):
                        nc.tensor.matmul(ps[:, :nsz], lhsT=w_sb2[:, c, :],
                                         rhs=x_sb[:, c, n0:n0 + nsz],
                                         start=(c == 0), stop=(c == 1))
                    nc.scalar.activation(dst[:, n0:n0 + nsz], ps[:, :nsz],
                                         AF.Identity, bias=b_sb2, scale=scale)
            for j in range(CT):
                pt = vt_ps.tile([128, HD], F32, tag="vtps")
                nc.tensor.transpose(pt, v_sb[:, 128 * j:128 * (j + 1)],
                                    ident[:HD, :HD])
                nc.scalar.copy(vTo_sb[:, j, :HD], pt)
        onecol_ap = bass.AP(tensor=ones_d, offset=0, ap=[[0, 128], [1, 1]])
        ones_col_f = const.tile([128, 1], F32)
        nc.sync.dma_start(out=ones_col_f, in_=onecol_ap)
        for j in range(CT):
            nc.vector.tensor_copy(vTo_sb[:, j, HD:HD + 1], ones_col_f)

        # ---- attention ----
        tokp = ctx.enter_context(tc.tile_pool(name="tok", bufs=2))
        yp = ctx.enter_context(tc.tile_pool(name="y", bufs=2 * NPAIRS + 1))
        ysp = ctx.enter_context(tc.tile_pool(name="ys", bufs=2 * NSING + 2))
        ep = ctx.enter_context(tc.tile_pool(name="e", bufs=4))
        rzp = ctx.enter_context(tc.tile_pool(name="rz", bufs=1))
        rzbp = ctx.enter_context(tc.tile_pool(name="rzb", bufs=2))
        ao_sb = qkv.tile([HD, N], F32)
        rz_sb = rzp.tile([1, N], F32)

        with tc.tile_pool(name="scps", bufs=2, space="PSUM") as sc_ps, \
             tc.tile_pool(name="avps", bufs=1, space="PSUM") as av_ps:
            avz = [av_ps.tile([HD + 1, 384], F32, tag=f"avz{s}", name=f"avz{s}")
                   for s in range(len(STRIPS))]
            for i in range(CT):
                c0 = 128 * i
                tokb = tokp.tile([128, N], FP16, tag="tok")
                nc.sync.dma_start(out=tokb, in_=tokT_d[c0:c0 + 128, :])

                for hf in range(NHALF):
                    h0 = hf * HW_
                    # scaled fp8 mask pairs (DoubleRow) + fp16 singles,
                    # half-width for build/consume double buffering
                    pair_tiles = []
                    for j in range(NPAIRS):
                        y = yp.tile([128, 2, HW_], FP8, tag="y")
                        eng = nc.gpsimd if j >= DVE_PAIRS else nc.vector
                        for half in range(2):
                            t = 2 * j + half
                            eng.tensor_scalar(out=y[:, half, :],
                                              in0=tokb[:, h0:h0 + HW_],
                                              scalar1=float(t),
                                              scalar2=emb_b[:, t:t + 1],
                                              op0=OP.is_equal, op1=OP.mult)
                        pair_tiles.append(y)
                    single_tiles = []
                    for t in range(2 * NPAIRS, NTERM):
                        y = ysp.tile([128, HW_], FP16, tag="ys")
                        nc.vector.tensor_scalar(out=y, in0=tokb[:, h0:h0 + HW_],
                                                scalar1=float(t),
                                                scalar2=emb_b[:, t:t + 1],
                                                op0=OP.is_equal, op1=OP.mult)
                        single_tiles.append(y)

                    for s, (p0, psz) in [(3 * hf + sl, STRIPS[3 * hf + sl])
                                         for sl in range(3)]:
                        q0 = p0 - h0
                        sc = sc_ps.tile([128, 384], F32, tag="sc")
                        nc.tensor.matmul(sc, lhsT=k_sb[:, c0:c0 + 128],
                                         rhs=q_sb[:, p0:p0 + psz],
                                         start=True, stop=True)
                        for y in pair_tiles:
                            nc.tensor.matmul(sc, lhsT=id8,
                                             rhs=y[:, :, q0:q0 + psz],
                                             start=False, stop=False,
                                             perf_mode=PM_DR,
                                             skip_group_check=True)
                        for y in single_tiles:
                            nc.tensor.matmul(sc, lhsT=identb,
                                             rhs=y[:, q0:q0 + psz],
                                             start=False, stop=False,
                                             skip_group_check=True)
                        e_sb = ep.tile([128, 384], FP16, tag="e")
                        nc.scalar.activation(e_sb, sc, AF.Exp)
                        nc.tensor.matmul(avz[s], lhsT=vTo_sb[:, i, :],
                                         rhs=e_sb,
                                         start=(i == 0), stop=(i == CT - 1))

            for s, (p0, psz) in enumerate(STRIPS):
                nc.scalar.copy(ao_sb[:, p0:p0 + psz], avz[s][:HD, :])
                nc.vector.reciprocal(rz_sb[:, p0:p0 + psz],
                                     avz[s][HD:HD + 1, :])

        # ---- output projection * rz + bias (fp16 partials) ----
        outp = ctx.enter_context(tc.tile_pool(name="outp", bufs=2))
        with tc.tile_pool(name="prps", bufs=2, space="PSUM") as pr_ps, \
             tc.tile_pool(name="rzbps", bufs=2, space="PSUM") as rzb_ps:
            for s, (p0, psz) in enumerate(STRIPS):
                rzb_psum = rzb_ps.tile([128, 384], F32, tag="rzbp")
                nc.tensor.matmul(rzb_psum[:, :psz], lhsT=ones_row,
                                 rhs=rz_sb[:, p0:p0 + psz], start=True, stop=True)
                rzb = rzbp.tile([128, 384], F32, tag="rzb")
                nc.scalar.copy(rzb[:, :psz], rzb_psum[:, :psz])
                for m in range(2):
                    pp = pr_ps.tile([128, 384], F32, tag="pr")
                    nc.tensor.matmul(pp[:, :psz],
                                     lhsT=wp_sb[:, 128 * m:128 * (m + 1)],
                                     rhs=ao_sb[:, p0:p0 + psz],
                                     start=True, stop=True)
                    ob = outp.tile([128, 384], FP16, tag="ob")
                    nc.vector.scalar_tensor_tensor(
                        out=ob[:, :psz], in0=pp[:, :psz],
                        scalar=1.0, in1=rzb[:, :psz],
                        op0=OP.mult, op1=OP.mult)
                    nc.vector.tensor_scalar(
                        out=ob[:, :psz], in0=ob[:, :psz],
                        scalar1=bp_sb[:, m:m + 1], scalar2=None, op0=OP.add)
                    nc.sync.dma_start(out_d[128 * m:128 * (m + 1), p0:p0 + psz],
                                      ob[:, :psz])

    nc.compile()
    return nc


_NC = None


def _get_nc():
    global _NC
    if _NC is None:
        _NC = build_nc()
    return _NC


def make_in_maps(x, Wq, bq, Wk, bk, Wv, bv, Wp, bp, emb, tokens):
    x_f = np.ascontiguousarray(np.asarray(x, np.float32).reshape(DIM, N))
    tokT = np.ascontiguousarray(np.asarray(tokens, np.int32).T.astype(np.float16))
    Wq, bq = np.asarray(Wq, np.float32), np.asarray(bq, np.float32)
    Wk, bk = np.asarray(Wk, np.float32), np.asarray(bk, np.float32)
    Wv, bv = np.asarray(Wv, np.float32), np.asarray(bv, np.float32)
    Wp, bp = np.asarray(Wp, np.float32), np.asarray(bp, np.float32)
    emb = np.asarray(emb, np.float32)
    ones128 = np.ones((1, 128), np.float32)

    in_maps = []
    for h in range(H):
        sl = slice(HD * h, HD * (h + 1))
        emb_col = emb[:NTERM, h].copy()
        in_maps.append({
            "x": x_f,
            "tokT": tokT,
            "wqT": np.ascontiguousarray(Wq[sl, :].T),
            "wkT": np.ascontiguousarray(Wk[sl, :].T),
            "wvT": np.ascontiguousarray(Wv[sl, :].T),
            "bq": np.ascontiguousarray((bq[sl] * SCALE).reshape(HD, 1)),
            "bk": np.ascontiguousarray(bk[sl].reshape(HD, 1)),
            "bv": np.ascontiguousarray(bv[sl].reshape(HD, 1)),
            "wpT": np.ascontiguousarray(Wp[:, sl].T),
            "bp": np.ascontiguousarray((bp / H).reshape(DIM, 1)),
            "embcol": np.ascontiguousarray(emb_col.reshape(1, NTERM)),
            "ones128": ones128,
        })
    return in_maps


def kernel(x, Wq, bq, Wk, bk, Wv, bv, Wp, bp, emb, tokens, _trace=False):
    nc = _get_nc()
    in_maps = make_in_maps(x, Wq, bq, Wk, bk, Wv, bv, Wp, bp, emb, tokens)
    res = run_bass_kernel_spmd(nc, in_maps, core_ids=list(range(H)), trace=_trace)
    out = np.zeros((DIM, N), np.float32)
    for r in res.results:
        out += np.asarray(r["out"], np.float32)
    ret = out.reshape(1, DIM, NX, NY)
    if _trace:
        return ret, res
    return ret



# revision 2
# speedup vs baseline: 90.9808x; 90.9808x over previous
"""Trainium2 Bass kernel for nn_Attention2D (2D attention with learnable
relative-position bias, attn_method=2 / pos_type=5).

Head-sharded (core h = head h); host sums the 8 partial projections.

Per core:
- tokens pre-transposed + pre-cast fp16 on host; HWDGE DMA only.
- scores TRANSPOSED: sc[c,p] = K_i^T Q_s per (key tile i, query strip s),
  contract over hd=32, fp16 operands.
- positional bias sum_t emb[t]*(tokT==t) built as fp16 masks on the DVE
  ONLY (GPSIMD tensor_scalar measured ~16.7us/op on HW - never use it),
  accumulated into the score PSUM via fp16 identity matmuls.
- SPECIALIZATION: when the runtime `tokens` input exactly equals the
  reference half-symmetric tokenization (checked on host), the program
  skips mask builds / bias matmuls for (tile, strip) blocks whose
  reference tokens are all padding, and emits per-block term subsets.
  Otherwise a fully general program (all 36 terms everywhere) runs.
- softmax: exp on ACT (unnormalized fp16); row-sum z rides the AV matmul
  as a ones column ([vT | 1] -> [33, 384] PSUM, row 32 = z).
  Renormalization happens after the output projection via a PE
  broadcast of rz = 1/z.
- partial projection output in fp16 (host sums in fp32).
"""

import numpy as np
from contextlib import ExitStack

import concourse.bacc as bacc
import concourse.bass as bass
import concourse.tile as tile
from concourse import mybir
from concourse.bass_utils import run_bass_kernel_spmd
from concourse.masks import make_identity

F32 = mybir.dt.float32
FP16 = mybir.dt.float16
AF = mybir.ActivationFunctionType
OP = mybir.AluOpType

DIM, H, HD = 256, 8, 32
NX = NY = 48
N = NX * NY            # 2304
RNG = 5
NTERM = 36             # non-pad bias table entries
PAD = NTERM
CT = N // 128          # 18 key tiles
SCALE = HD ** -0.5

STRIPS = [(i * 384, 384) for i in range(N // 384)]          # 6 x 384
NHALF = 2
HW_ = N // NHALF                                             # 1152
MASK_BUFS = 60         # fp16 single-term mask buffers (>=36 for 1 half)


def _make_tokens_ref():
    xm, ym = np.meshgrid(np.arange(NX), np.arange(NY), indexing='ij')
    xm, ym = xm.ravel(), ym.ravel()
    dx = xm[None, :] - xm[:, None]
    dy = ym[None, :] - ym[:, None]
    tok = np.abs(dx) * (RNG + 1) + np.abs(dy)
    tok[(np.abs(dx) > RNG) | (np.abs(dy) > RNG)] = PAD
    return tok.astype(np.int32)


TOK_REF = _make_tokens_ref()


def _term_plan(specialize):
    """Per (key tile, half): {term: (col0, width)} build spans; per
    (key tile, strip): term list to consume.

    specialize=None -> all 36 terms everywhere (general program).
    Otherwise specialize is the [k, q] token matrix the plan is built for.
    The build span of a term within a half covers exactly the strips of
    that half in which the term occurs (contiguous min..max strip range),
    so the DVE only touches columns that can be consumed.
    """
    half_terms = {}
    strip_terms = {}
    for i in range(CT):
        for s, (p0, psz) in enumerate(STRIPS):
            if specialize is None:
                strip_terms[(i, s)] = list(range(NTERM))
            else:
                blk = specialize[128 * i:128 * (i + 1), p0:p0 + psz]
                strip_terms[(i, s)] = sorted(set(np.unique(blk)) - {PAD})
        for hf in range(NHALF):
            spans = {}
            for sl in range(3):
                for t in strip_terms[(i, 3 * hf + sl)]:
                    lo, hi = spans.get(t, (sl, sl))
                    spans[t] = (min(lo, sl), max(hi, sl))
            half_terms[(i, hf)] = {
                t: (lo * 384, (hi - lo + 1) * 384)
                for t, (lo, hi) in sorted(spans.items())
            }
    return half_terms, strip_terms


def build_nc(specialize=None):
    half_terms, strip_terms = _term_plan(specialize)

    nc = bacc.Bacc("TRN2", target_bir_lowering=False)

    x_d = nc.dram_tensor("x", [DIM, N], FP16, kind="ExternalInput")
    tokT_d = nc.dram_tensor("tokT", [N, N], FP16, kind="ExternalInput")
    wqT_d = nc.dram_tensor("wqT", [DIM, HD], FP16, kind="ExternalInput")
    wkT_d = nc.dram_tensor("wkT", [DIM, HD], FP16, kind="ExternalInput")
    wvT_d = nc.dram_tensor("wvT", [DIM, HD], FP16, kind="ExternalInput")
    bq_d = nc.dram_tensor("bq", [HD, 1], F32, kind="ExternalInput")   # pre *SCALE
    bk_d = nc.dram_tensor("bk", [HD, 1], F32, kind="ExternalInput")
    bv_d = nc.dram_tensor("bv", [HD, 1], F32, kind="ExternalInput")
    wpT_d = nc.dram_tensor("wpT", [HD, DIM], FP16, kind="ExternalInput")
    bp_d = nc.dram_tensor("bp", [DIM, 1], F32, kind="ExternalInput")  # pre /8
    emb_d = nc.dram_tensor("embcol", [1, NTERM], F32, kind="ExternalInput")
    ones_d = nc.dram_tensor("ones128", [1, 128], F32, kind="ExternalInput")
    out_d = nc.dram_tensor("out", [DIM, N], FP16, kind="ExternalOutput")

    with tile.TileContext(nc) as tc, ExitStack() as ctx:
        const = ctx.enter_context(tc.tile_pool(name="const", bufs=1))

        ident = const.tile([128, 128], F32)
        make_identity(nc, ident)
        identb = const.tile([128, 128], FP16)
        nc.vector.tensor_copy(identb, ident)

        emb_b = const.tile([128, NTERM], F32)
        eap = emb_d[0:1, :]
        nc.sync.dma_start(
            out=emb_b,
            in_=bass.AP(tensor=eap.tensor, offset=eap.offset,
                        ap=[[0, 128], [1, NTERM]]),
        )
        ones_row = const.tile([1, 128], F32)
        nc.sync.dma_start(out=ones_row, in_=ones_d[0:1, :])

        # ---- weights ----
        xw = ctx.enter_context(tc.tile_pool(name="xw", bufs=1))
        wq_sb = xw.tile([128, 2, HD], FP16)
        wk_sb = xw.tile([128, 2, HD], FP16)
        wv_sb = xw.tile([128, 2, HD], FP16)
        for w_sb, w_dr in ((wq_sb, wqT_d), (wk_sb, wkT_d), (wv_sb, wvT_d)):
            for c in range(2):
                nc.sync.dma_start(out=w_sb[:, c, :], in_=w_dr[128 * c:128 * (c + 1), :])
        bq_sb = xw.tile([HD, 1], F32)
        bk_sb = xw.tile([HD, 1], F32)
        bv_sb = xw.tile([HD, 1], F32)
        for b_sb, b_dr in ((bq_sb, bq_d), (bk_sb, bk_d), (bv_sb, bv_d)):
            nc.sync.dma_start(out=b_sb, in_=b_dr[:, :])
        wp_sb = xw.tile([HD, DIM], FP16)
        nc.sync.dma_start(out=wp_sb, in_=wpT_d[:, :])
        bp_sb = xw.tile([128, 2], F32)
        for m in range(2):
            nc.sync.dma_start(out=bp_sb[:, m:m + 1], in_=bp_d[128 * m:128 * (m + 1), :])

        # ---- q/k/v projections (q/k in fp16); vT with ones column ----
        qkv = ctx.enter_context(tc.tile_pool(name="qkv", bufs=1))
        q_sb = qkv.tile([HD, N], FP16)
        k_sb = qkv.tile([HD, N], FP16)
        vTo_sb = qkv.tile([128, CT, HD + 1], FP16)
        nc.vector.memset(vTo_sb, 0.0)

        with tc.tile_pool(name="xv", bufs=1) as xv, \
             tc.tile_pool(name="qkvp", bufs=2, space="PSUM") as qkv_ps, \
             tc.tile_pool(name="vtp", bufs=2, space="PSUM") as vt_ps:
            x_sb = xv.tile([128, 2, N], FP16)
            for c in range(2):
                nc.sync.dma_start(out=x_sb[:, c, :],
                                  in_=x_d[128 * c:128 * (c + 1), :])
            v_sb = xv.tile([HD, N], F32)
            for dst, w_sb2, b_sb2, scale in (
                (q_sb, wq_sb, bq_sb, SCALE),
                (k_sb, wk_sb, bk_sb, 1.0),
                (v_sb, wv_sb, bv_sb, 1.0),
            ):
                for n0, nsz in STRIPS:
                    ps = qkv_ps.tile([HD, 512], F32, tag="qkvps")
                    for c in range(2):
                        nc.tensor.matmul(ps[:, :nsz], lhsT=w_sb2[:, c, :],
                                         rhs=x_sb[:, c, n0:n0 + nsz],
                                         start=(c == 0), stop=(c == 1))
                    nc.scalar.activation(dst[:, n0:n0 + nsz], ps[:, :nsz],
                                         AF.Identity, bias=b_sb2, scale=scale)
            for j in range(CT):
                pt = vt_ps.tile([128, HD], F32, tag="vtps")
                nc.tensor.transpose(pt, v_sb[:, 128 * j:128 * (j + 1)],
                                    ident[:HD, :HD])
                nc.scalar.copy(vTo_sb[:, j, :HD], pt)
        onecol_ap = bass.AP(tensor=ones_d, offset=0, ap=[[0, 128], [1, 1]])
        ones_col_f = const.tile([128, 1], F32)
        nc.sync.dma_start(out=ones_col_f, in_=onecol_ap)
        for j in range(CT):
            nc.vector.tensor_copy(vTo_sb[:, j, HD:HD + 1], ones_col_f)

        # ---- attention ----
        tokp = ctx.enter_context(tc.tile_pool(name="tok", bufs=2))
        ysp = ctx.enter_context(tc.tile_pool(name="ys", bufs=MASK_BUFS))
        ep = ctx.enter_context(tc.tile_pool(name="e", bufs=4))
        rzp = ctx.enter_context(tc.tile_pool(name="rz", bufs=1))
        rzbp = ctx.enter_context(tc.tile_pool(name="rzb", bufs=2))
        ao_sb = qkv.tile([HD, N], FP16)
        rz_sb = rzp.tile([1, N], F32)

        with tc.tile_pool(name="scps", bufs=2, space="PSUM") as sc_ps, \
             tc.tile_pool(name="avps", bufs=1, space="PSUM") as av_ps:
            avz = [av_ps.tile([HD + 1, 384], F32, tag=f"avz{s}", name=f"avz{s}")
                   for s in range(len(STRIPS))]
            for i in range(CT):
                c0 = 128 * i
                tokb = tokp.tile([128, N], FP16, tag="tok")
                nc.sync.dma_start(out=tokb, in_=tokT_d[c0:c0 + 128, :])

                for hf in range(NHALF):
                    h0 = hf * HW_
                    ytile = {}
                    for t, (b0, bw) in half_terms[(i, hf)].items():
                        y = ysp.tile([128, HW_], FP16, tag="ys")
                        nc.vector.tensor_scalar(out=y[:, b0:b0 + bw],
                                                in0=tokb[:, h0 + b0:h0 + b0 + bw],
                                                scalar1=float(t),
                                                scalar2=emb_b[:, t:t + 1],
                                                op0=OP.is_equal, op1=OP.mult)
                        ytile[t] = y

                    for s, (p0, psz) in [(3 * hf + sl, STRIPS[3 * hf + sl])
                                         for sl in range(3)]:
                        q0 = p0 - h0
                        sc = sc_ps.tile([128, 384], F32, tag="sc")
                        nc.tensor.matmul(sc, lhsT=k_sb[:, c0:c0 + 128],
                                         rhs=q_sb[:, p0:p0 + psz],
                                         start=True, stop=True)
                        for t in strip_terms[(i, s)]:
                            nc.tensor.matmul(sc, lhsT=identb,
                                             rhs=ytile[t][:, q0:q0 + psz],
                                             start=False, stop=False,
                                             skip_group_check=True)
                        e_sb = ep.tile([128, 384], FP16, tag="e")
                        nc.scalar.activation(e_sb, sc, AF.Exp)
                        nc.tensor.matmul(avz[s], lhsT=vTo_sb[:, i, :],
                                         rhs=e_sb,
                                         start=(i == 0), stop=(i == CT - 1))

            for s, (p0, psz) in enumerate(STRIPS):
                nc.scalar.copy(ao_sb[:, p0:p0 + psz], avz[s][:HD, :])
                nc.vector.reciprocal(rz_sb[:, p0:p0 + psz],
                                     avz[s][HD:HD + 1, :])

        # ---- output projection * rz + bias (fp16 partials) ----
        outp = ctx.enter_context(tc.tile_pool(name="outp", bufs=2))
        with tc.tile_pool(name="prps", bufs=2, space="PSUM") as pr_ps, \
             tc.tile_pool(name="rzbps", bufs=2, space="PSUM") as rzb_ps:
            for s, (p0, psz) in enumerate(STRIPS):
                rzb_psum = rzb_ps.tile([128, 384], F32, tag="rzbp")
                nc.tensor.matmul(rzb_psum[:, :psz], lhsT=ones_row,
                                 rhs=rz_sb[:, p0:p0 + psz], start=True, stop=True)
                rzb = rzbp.tile([128, 384], F32, tag="rzb")
                nc.scalar.copy(rzb[:, :psz], rzb_psum[:, :psz])
                for m in range(2):
                    pp = pr_ps.tile([128, 384], F32, tag="pr")
                    nc.tensor.matmul(pp[:, :psz],
                                     lhsT=wp_sb[:, 128 * m:128 * (m + 1)],
                                     rhs=ao_sb[:, p0:p0 + psz],
                                     start=True, stop=True)
                    ob = outp.tile([128, 384], FP16, tag="ob")
                    nc.vector.scalar_tensor_tensor(
                        out=ob[:, :psz], in0=pp[:, :psz],
                        scalar=1.0, in1=rzb[:, :psz],
                        op0=OP.mult, op1=OP.mult)
                    nc.vector.tensor_scalar(
                        out=ob[:, :psz], in0=ob[:, :psz],
                        scalar1=bp_sb[:, m:m + 1], scalar2=None, op0=OP.add)
                    nc.sync.dma_start(out_d[128 * m:128 * (m + 1), p0:p0 + psz],
                                      ob[:, :psz])

    nc.compile()
    return nc


_NC = {}


def _get_nc(spec):
    key = "spec" if spec else "gen"
    if key not in _NC:
        _NC[key] = build_nc(specialize=TOK_REF if spec else None)
    return _NC[key]


def make_in_maps(x, Wq, bq, Wk, bk, Wv, bv, Wp, bp, emb, tokens):
    x_f = np.ascontiguousarray(np.asarray(x, np.float32).reshape(DIM, N).astype(np.float16))
    tokT = np.ascontiguousarray(np.asarray(tokens, np.int32).T.astype(np.float16))
    Wq, bq = np.asarray(Wq, np.float32), np.asarray(bq, np.float32)
    Wk, bk = np.asarray(Wk, np.float32), np.asarray(bk, np.float32)
    Wv, bv = np.asarray(Wv, np.float32), np.asarray(bv, np.float32)
    Wp, bp = np.asarray(Wp, np.float32), np.asarray(bp, np.float32)
    emb = np.asarray(emb, np.float32)
    ones128 = np.ones((1, 128), np.float32)

    in_maps = []
    for h in range(H):
        sl = slice(HD * h, HD * (h + 1))
        emb_col = emb[:NTERM, h].copy()
        in_maps.append({
            "x": x_f,
            "tokT": tokT,
            "wqT": np.ascontiguousarray(Wq[sl, :].T.astype(np.float16)),
            "wkT": np.ascontiguousarray(Wk[sl, :].T.astype(np.float16)),
            "wvT": np.ascontiguousarray(Wv[sl, :].T.astype(np.float16)),
            "bq": np.ascontiguousarray((bq[sl] * SCALE).reshape(HD, 1)),
            "bk": np.ascontiguousarray(bk[sl].reshape(HD, 1)),
            "bv": np.ascontiguousarray(bv[sl].reshape(HD, 1)),
            "wpT": np.ascontiguousarray(Wp[:, sl].T.astype(np.float16)),
            "bp": np.ascontiguousarray((bp / H).reshape(DIM, 1)),
            "embcol": np.ascontiguousarray(emb_col.reshape(1, NTERM)),
            "ones128": ones128,
        })
    return in_maps


def kernel(x, Wq, bq, Wk, bk, Wv, bv, Wp, bp, emb, tokens, _trace=False):
    spec = np.array_equal(np.asarray(tokens, np.int32), TOK_REF)
    nc = _get_nc(spec)
    in_maps = make_in_maps(x, Wq, bq, Wk, bk, Wv, bv, Wp, bp, emb, tokens)
    res = run_bass_kernel_spmd(nc, in_maps, core_ids=list(range(H)), trace=_trace)
    out = np.zeros((DIM, N), np.float32)
    for r in res.results:
        out += np.asarray(r["out"], np.float32)
    ret = out.reshape(1, DIM, NX, NY)
    if _trace:
        return ret, res
    return ret


# revision 3
# speedup vs baseline: 177.0192x; 1.9457x over previous
"""Trainium2 Bass kernel for nn_Attention2D (2D attention with learnable
relative-position bias, attn_method=2 / pos_type=5).

Head-sharded (core h = head h); host sums the 8 partial projections.

Per core:
- tokens pre-transposed + pre-cast fp16 on host; HWDGE DMA only.
- scores TRANSPOSED: sc[c,p] = K_i^T Q_s per (key tile i, query strip s),
  contract over hd=32, fp16 operands.
- positional bias sum_t emb[t]*(tokT==t) built as fp16 masks on the DVE
  ONLY (GPSIMD tensor_scalar measured ~16.7us/op on HW - never use it),
  accumulated into the score PSUM via fp16 identity matmuls.
- SPECIALIZATION: when the runtime `tokens` input exactly equals the
  reference half-symmetric tokenization (checked on host), the program
  skips mask builds / bias matmuls for (tile, strip) blocks whose
  reference tokens are all padding, and emits per-block term subsets.
  Otherwise a fully general program (all 36 terms everywhere) runs.
- softmax: exp on ACT (unnormalized fp16); row-sum z rides the AV matmul
  as a ones column ([vT | 1] -> [33, 384] PSUM, row 32 = z).
  Renormalization happens after the output projection via a PE
  broadcast of rz = 1/z.
- partial projection output in fp16 (host sums in fp32).
"""

import numpy as np
from contextlib import ExitStack

import concourse.bacc as bacc
import concourse.bass as bass
import concourse.tile as tile
from concourse import mybir
from concourse.bass_utils import run_bass_kernel_spmd
from concourse.masks import make_identity

F32 = mybir.dt.float32
FP16 = mybir.dt.float16
AF = mybir.ActivationFunctionType
OP = mybir.AluOpType

DIM, H, HD = 256, 8, 32
NX = NY = 48
N = NX * NY            # 2304
RNG = 5
NTERM = 36             # non-pad bias table entries
PAD = NTERM
CT = N // 128          # 18 key tiles
SCALE = HD ** -0.5

STRIPS = [(i * 384, 384) for i in range(N // 384)]          # 6 x 384
NHALF = 2
HW_ = N // NHALF                                             # 1152
MASK_BUFS = 60         # fp16 single-term mask buffers (>=36 for 1 half)


def _make_tokens_ref():
    xm, ym = np.meshgrid(np.arange(NX), np.arange(NY), indexing='ij')
    xm, ym = xm.ravel(), ym.ravel()
    dx = xm[None, :] - xm[:, None]
    dy = ym[None, :] - ym[:, None]
    tok = np.abs(dx) * (RNG + 1) + np.abs(dy)
    tok[(np.abs(dx) > RNG) | (np.abs(dy) > RNG)] = PAD
    return tok.astype(np.int32)


TOK_REF = _make_tokens_ref()


def _term_plan(specialize):
    """Per (key tile, half): {term: (col0, width)} build spans; per
    (key tile, strip): term list to consume.

    specialize=None -> all 36 terms everywhere (general program).
    Otherwise specialize is the [k, q] token matrix the plan is built for.
    The build span of a term within a half covers exactly the strips of
    that half in which the term occurs (contiguous min..max strip range),
    so the DVE only touches columns that can be consumed.
    """
    half_terms = {}
    strip_terms = {}
    for i in range(CT):
        blk = None if specialize is None else specialize[128 * i:128 * (i + 1), :]
        for s, (p0, psz) in enumerate(STRIPS):
            if specialize is None:
                strip_terms[(i, s)] = {t: (0, psz) for t in range(NTERM)}
            else:
                sub = blk[:, p0:p0 + psz]
                d = {}
                for t in sorted(set(np.unique(sub)) - {PAD}):
                    cols = np.where((sub == t).any(axis=0))[0]
                    d[t] = (int(cols.min()), int(cols.max() - cols.min() + 1))
                strip_terms[(i, s)] = d
        for hf in range(NHALF):
            if specialize is None:
                half_terms[(i, hf)] = {t: (0, HW_) for t in range(NTERM)}
                continue
            sub = blk[:, hf * HW_:(hf + 1) * HW_]
            d = {}
            for t in sorted(set(np.unique(sub)) - {PAD}):
                cols = np.where((sub == t).any(axis=0))[0]
                d[t] = (int(cols.min()), int(cols.max() - cols.min() + 1))
            half_terms[(i, hf)] = d
    return half_terms, strip_terms


def build_nc(specialize=None):
    half_terms, strip_terms = _term_plan(specialize)

    nc = bacc.Bacc("TRN2", target_bir_lowering=False)

    x_d = nc.dram_tensor("x", [DIM, N], FP16, kind="ExternalInput")
    tokT_d = nc.dram_tensor("tokT", [N, N], FP16, kind="ExternalInput")
    wqT_d = nc.dram_tensor("wqT", [DIM, HD], FP16, kind="ExternalInput")
    wkT_d = nc.dram_tensor("wkT", [DIM, HD], FP16, kind="ExternalInput")
    wvT_d = nc.dram_tensor("wvT", [DIM, HD], FP16, kind="ExternalInput")
    bq_d = nc.dram_tensor("bq", [HD, 1], F32, kind="ExternalInput")   # pre *SCALE
    bk_d = nc.dram_tensor("bk", [HD, 1], F32, kind="ExternalInput")
    bv_d = nc.dram_tensor("bv", [HD, 1], F32, kind="ExternalInput")
    wpT_d = nc.dram_tensor("wpT", [HD, DIM], FP16, kind="ExternalInput")
    bp_d = nc.dram_tensor("bp", [DIM, 1], F32, kind="ExternalInput")  # pre /8
    emb_d = nc.dram_tensor("embcol", [1, NTERM], F32, kind="ExternalInput")
    ones_d = nc.dram_tensor("ones128", [1, 128], F32, kind="ExternalInput")
    out_d = nc.dram_tensor("out", [DIM, N], FP16, kind="ExternalOutput")

    with tile.TileContext(nc) as tc, ExitStack() as ctx:
        const = ctx.enter_context(tc.tile_pool(name="const", bufs=1))

        ident = const.tile([128, 128], F32)
        make_identity(nc, ident)
        identb = const.tile([128, 128], FP16)
        nc.vector.tensor_copy(identb, ident)

        emb_b = const.tile([128, NTERM], F32)
        eap = emb_d[0:1, :]
        nc.sync.dma_start(
            out=emb_b,
            in_=bass.AP(tensor=eap.tensor, offset=eap.offset,
                        ap=[[0, 128], [1, NTERM]]),
        )
        ones_row = const.tile([1, 128], F32)
        nc.sync.dma_start(out=ones_row, in_=ones_d[0:1, :])

        # ---- weights ----
        xw = ctx.enter_context(tc.tile_pool(name="xw", bufs=1))
        wq_sb = xw.tile([128, 2, HD], FP16)
        wk_sb = xw.tile([128, 2, HD], FP16)
        wv_sb = xw.tile([128, 2, HD], FP16)
        for w_sb, w_dr in ((wq_sb, wqT_d), (wk_sb, wkT_d), (wv_sb, wvT_d)):
            for c in range(2):
                nc.sync.dma_start(out=w_sb[:, c, :], in_=w_dr[128 * c:128 * (c + 1), :])
        bq_sb = xw.tile([HD, 1], F32)
        bk_sb = xw.tile([HD, 1], F32)
        bv_sb = xw.tile([HD, 1], F32)
        for b_sb, b_dr in ((bq_sb, bq_d), (bk_sb, bk_d), (bv_sb, bv_d)):
            nc.sync.dma_start(out=b_sb, in_=b_dr[:, :])
        wp_sb = xw.tile([HD, DIM], FP16)
        nc.sync.dma_start(out=wp_sb, in_=wpT_d[:, :])
        bp_sb = xw.tile([128, 2], F32)
        for m in range(2):
            nc.sync.dma_start(out=bp_sb[:, m:m + 1], in_=bp_d[128 * m:128 * (m + 1), :])

        # ---- q/k/v projections (q/k in fp16); vT with ones column ----
        qkv = ctx.enter_context(tc.tile_pool(name="qkv", bufs=1))
        q_sb = qkv.tile([HD, N], FP16)
        k_sb = qkv.tile([HD, N], FP16)
        vTo_sb = qkv.tile([128, CT, HD + 1], FP16)
        nc.vector.memset(vTo_sb, 0.0)

        with tc.tile_pool(name="xv", bufs=1) as xv, \
             tc.tile_pool(name="qkvp", bufs=2, space="PSUM") as qkv_ps, \
             tc.tile_pool(name="vtp", bufs=2, space="PSUM") as vt_ps:
            x_sb = xv.tile([128, 2, N], FP16)
            for c in range(2):
                nc.sync.dma_start(out=x_sb[:, c, :],
                                  in_=x_d[128 * c:128 * (c + 1), :])
            v_sb = xv.tile([HD, N], F32)
            for dst, w_sb2, b_sb2, scale in (
                (q_sb, wq_sb, bq_sb, SCALE),
                (k_sb, wk_sb, bk_sb, 1.0),
                (v_sb, wv_sb, bv_sb, 1.0),
            ):
                for n0, nsz in STRIPS:
                    ps = qkv_ps.tile([HD, 512], F32, tag="qkvps")
                    for c in range(2):
                        nc.tensor.matmul(ps[:, :nsz], lhsT=w_sb2[:, c, :],
                                         rhs=x_sb[:, c, n0:n0 + nsz],
                                         start=(c == 0), stop=(c == 1))
                    nc.scalar.activation(dst[:, n0:n0 + nsz], ps[:, :nsz],
                                         AF.Identity, bias=b_sb2, scale=scale)
            for j in range(CT):
                pt = vt_ps.tile([128, HD], F32, tag="vtps")
                nc.tensor.transpose(pt, v_sb[:, 128 * j:128 * (j + 1)],
                                    ident[:HD, :HD])
                nc.scalar.copy(vTo_sb[:, j, :HD], pt)
        onecol_ap = bass.AP(tensor=ones_d, offset=0, ap=[[0, 128], [1, 1]])
        ones_col_f = const.tile([128, 1], F32)
        nc.sync.dma_start(out=ones_col_f, in_=onecol_ap)
        for j in range(CT):
            nc.vector.tensor_copy(vTo_sb[:, j, HD:HD + 1], ones_col_f)

        # ---- attention ----
        tokp = ctx.enter_context(tc.tile_pool(name="tok", bufs=2))
        ysp = ctx.enter_context(tc.tile_pool(name="ys", bufs=MASK_BUFS))
        ep = ctx.enter_context(tc.tile_pool(name="e", bufs=4))
        rzp = ctx.enter_context(tc.tile_pool(name="rz", bufs=1))
        rzbp = ctx.enter_context(tc.tile_pool(name="rzb", bufs=2))
        ao_sb = qkv.tile([HD, N], FP16)
        rz_sb = rzp.tile([1, N], F32)

        with tc.tile_pool(name="scps", bufs=2, space="PSUM") as sc_ps, \
             tc.tile_pool(name="avps", bufs=1, space="PSUM") as av_ps:
            avz = [av_ps.tile([HD + 1, 384], F32, tag=f"avz{s}", name=f"avz{s}")
                   for s in range(len(STRIPS))]
            for i in range(CT):
                c0 = 128 * i
                tokb = tokp.tile([128, N], FP16, tag="tok")
                nc.sync.dma_start(out=tokb, in_=tokT_d[c0:c0 + 128, :])

                for hf in range(NHALF):
                    h0 = hf * HW_
                    ytile = {}
                    for t, (b0, bw) in half_terms[(i, hf)].items():
                        y = ysp.tile([128, HW_], FP16, tag="ys")
                        nc.vector.tensor_scalar(out=y[:, b0:b0 + bw],
                                                in0=tokb[:, h0 + b0:h0 + b0 + bw],
                                                scalar1=float(t),
                                                scalar2=emb_b[:, t:t + 1],
                                                op0=OP.is_equal, op1=OP.mult)
                        ytile[t] = y

                    for s, (p0, psz) in [(3 * hf + sl, STRIPS[3 * hf + sl])
                                         for sl in range(3)]:
                        q0 = p0 - h0
                        sc = sc_ps.tile([128, 384], F32, tag="sc")
                        nc.tensor.matmul(sc, lhsT=k_sb[:, c0:c0 + 128],
                                         rhs=q_sb[:, p0:p0 + psz],
                                         start=True, stop=True)
                        for t, (ca, cw) in strip_terms[(i, s)].items():
                            # ca is strip-local; build hull is half-local and
                            # always covers [p0-h0+ca, p0-h0+ca+cw).
                            nc.tensor.matmul(sc[:, ca:ca + cw], lhsT=identb,
                                             rhs=ytile[t][:, q0 + ca:q0 + ca + cw],
                                             start=False, stop=False,
                                             skip_group_check=True)
                        e_sb = ep.tile([128, 384], FP16, tag="e")
                        nc.scalar.activation(e_sb, sc, AF.Exp)
                        nc.tensor.matmul(avz[s], lhsT=vTo_sb[:, i, :],
                                         rhs=e_sb,
                                         start=(i == 0), stop=(i == CT - 1))

            for s, (p0, psz) in enumerate(STRIPS):
                nc.scalar.copy(ao_sb[:, p0:p0 + psz], avz[s][:HD, :])
                nc.vector.reciprocal(rz_sb[:, p0:p0 + psz],
                                     avz[s][HD:HD + 1, :])

        # ---- output projection * rz + bias (fp16 partials) ----
        outp = ctx.enter_context(tc.tile_pool(name="outp", bufs=2))
        with tc.tile_pool(name="prps", bufs=2, space="PSUM") as pr_ps, \
             tc.tile_pool(name="rzbps", bufs=2, space="PSUM") as rzb_ps:
            for s, (p0, psz) in enumerate(STRIPS):
                rzb_psum = rzb_ps.tile([128, 384], F32, tag="rzbp")
                nc.tensor.matmul(rzb_psum[:, :psz], lhsT=ones_row,
                                 rhs=rz_sb[:, p0:p0 + psz], start=True, stop=True)
                rzb = rzbp.tile([128, 384], F32, tag="rzb")
                nc.scalar.copy(rzb[:, :psz], rzb_psum[:, :psz])
                for m in range(2):
                    pp = pr_ps.tile([128, 384], F32, tag="pr")
                    nc.tensor.matmul(pp[:, :psz],
                                     lhsT=wp_sb[:, 128 * m:128 * (m + 1)],
                                     rhs=ao_sb[:, p0:p0 + psz],
                                     start=True, stop=True)
                    ob = outp.tile([128, 384], FP16, tag="ob")
                    nc.vector.scalar_tensor_tensor(
                        out=ob[:, :psz], in0=pp[:, :psz],
                        scalar=1.0, in1=rzb[:, :psz],
                        op0=OP.mult, op1=OP.mult)
                    nc.vector.tensor_scalar(
                        out=ob[:, :psz], in0=ob[:, :psz],
                        scalar1=bp_sb[:, m:m + 1], scalar2=None, op0=OP.add)
                    nc.sync.dma_start(out_d[128 * m:128 * (m + 1), p0:p0 + psz],
                                      ob[:, :psz])

    nc.compile()
    return nc


_NC = {}


def _get_nc(spec):
    key = "spec" if spec else "gen"
    if key not in _NC:
        _NC[key] = build_nc(specialize=TOK_REF if spec else None)
    return _NC[key]


def make_in_maps(x, Wq, bq, Wk, bk, Wv, bv, Wp, bp, emb, tokens):
    x_f = np.ascontiguousarray(np.asarray(x, np.float32).reshape(DIM, N).astype(np.float16))
    tokT = np.ascontiguousarray(np.asarray(tokens, np.int32).T.astype(np.float16))
    Wq, bq = np.asarray(Wq, np.float32), np.asarray(bq, np.float32)
    Wk, bk = np.asarray(Wk, np.float32), np.asarray(bk, np.float32)
    Wv, bv = np.asarray(Wv, np.float32), np.asarray(bv, np.float32)
    Wp, bp = np.asarray(Wp, np.float32), np.asarray(bp, np.float32)
    emb = np.asarray(emb, np.float32)
    ones128 = np.ones((1, 128), np.float32)

    in_maps = []
    for h in range(H):
        sl = slice(HD * h, HD * (h + 1))
        emb_col = emb[:NTERM, h].copy()
        in_maps.append({
            "x": x_f,
            "tokT": tokT,
            "wqT": np.ascontiguousarray(Wq[sl, :].T.astype(np.float16)),
            "wkT": np.ascontiguousarray(Wk[sl, :].T.astype(np.float16)),
            "wvT": np.ascontiguousarray(Wv[sl, :].T.astype(np.float16)),
            "bq": np.ascontiguousarray((bq[sl] * SCALE).reshape(HD, 1)),
            "bk": np.ascontiguousarray(bk[sl].reshape(HD, 1)),
            "bv": np.ascontiguousarray(bv[sl].reshape(HD, 1)),
            "wpT": np.ascontiguousarray(Wp[:, sl].T.astype(np.float16)),
            "bp": np.ascontiguousarray((bp / H).reshape(DIM, 1)),
            "embcol": np.ascontiguousarray(emb_col.reshape(1, NTERM)),
            "ones128": ones128,
        })
    return in_maps


def kernel(x, Wq, bq, Wk, bk, Wv, bv, Wp, bp, emb, tokens, _trace=False):
    spec = np.array_equal(np.asarray(tokens, np.int32), TOK_REF)
    nc = _get_nc(spec)
    in_maps = make_in_maps(x, Wq, bq, Wk, bk, Wv, bv, Wp, bp, emb, tokens)
    res = run_bass_kernel_spmd(nc, in_maps, core_ids=list(range(H)), trace=_trace)
    out = np.zeros((DIM, N), np.float32)
    for r in res.results:
        out += np.asarray(r["out"], np.float32)
    ret = out.reshape(1, DIM, NX, NY)
    if _trace:
        return ret, res
    return ret
